# revision 1
# baseline (speedup 1.0000x reference)
"""Fused multi-head-size-1 attention kernel for Trainium2 (Bass/Tile).

Problem: out = softmax((x_q Wq^T + bq)(x_k Wk^T + bk)^T / sqrt(D)) (x_v Wv^T + bv)
Shapes: B=8, QL=KL=2048, D=1024, fp32 in/out.

Sharding: data-parallel over batch. Core i processes batch i end-to-end;
no collectives. Host pre-transposes x/W to contraction-major layout and
casts matmul operands to bf16 (PE runs bf16 at 1 cycle/row vs 4 for fp32;
all accumulation stays fp32 in PSUM).

Per-core dataflow (everything resident in SBUF in bf16):
  phase 1: K^T[h,k'] = Wk @ xk^T (+bk), V[k',h] = xv @ Wv^T (ones col
           appended for the softmax denominator), Q^T[h,q] = Wq @ xq^T (+bq)
  phase 2: per q-block: S^T[k',q] = K Q^T (PSUM, fp32), P^T = exp(S^T/32)
           (ScalarE, bf16 out), O[q,h] (+l) = P V_aug (PSUM, fp32),
           O = O * (1/l) + bv, DMA out.
"""

import numpy as np
import ml_dtypes

import concourse.bass as bass
import concourse.mybir as mybir
from concourse.bacc import Bacc
from concourse.tile import TileContext
from concourse.bass_utils import run_bass_kernel_spmd

B, QL, KL, D = 8, 2048, 2048, 1024
P = 128
NCORES = 8
DT = D // P          # 8 tiles along d/h
KT = KL // P         # 16 tiles along k'
XCH = 512            # x streaming chunk along s
QB = 512             # q block for the attention stage
F32 = mybir.dt.float32
BF16 = mybir.dt.bfloat16
SCALE = 1.0 / 32.0   # 1/sqrt(D)

# AV free-dim chunking over V's 1025 columns (1024 h + ones column for l).
# The l-carrying chunk goes first so the reciprocal overlaps the other
# chunks' matmuls.
AV_CHUNKS = [(684, 1025), (0, 342), (342, 684)]
AV_MAXW = 342


def build_bass() -> bass.Bass:
    # Bacc (not bare Bass): its finalize() runs the pass pipeline that splits
    # multi-semaphore waits into event semaphores (TRN2 allows 1 wait/inst).
    nc = Bacc()

    xqT = nc.declare_dram_parameter("xqT", [D, QL], BF16, isOutput=False)
    xkT = nc.declare_dram_parameter("xkT", [D, KL], BF16, isOutput=False)
    xvT = nc.declare_dram_parameter("xvT", [D, KL], BF16, isOutput=False)
    wqT = nc.declare_dram_parameter("wqT", [D, D], BF16, isOutput=False)
    wkT = nc.declare_dram_parameter("wkT", [D, D], BF16, isOutput=False)
    wvT = nc.declare_dram_parameter("wvT", [D, D], BF16, isOutput=False)
    bqp = nc.declare_dram_parameter("bqp", [P, DT], F32, isOutput=False)
    bkp = nc.declare_dram_parameter("bkp", [P, DT], F32, isOutput=False)
    bv = nc.declare_dram_parameter("bv", [D], F32, isOutput=False)
    out = nc.declare_dram_parameter("out", [QL, D], F32, isOutput=True)

    # contraction-major views: d = dt*128 + p
    xq_r = xqT[:].rearrange("(dt p) s -> p dt s", p=P)
    xk_r = xkT[:].rearrange("(dt p) s -> p dt s", p=P)
    xv_r = xvT[:].rearrange("(dt p) s -> p dt s", p=P)
    wq_r = wqT[:].rearrange("(dt p) h -> p dt h", p=P)
    wk_r = wkT[:].rearrange("(dt p) h -> p dt h", p=P)
    wv_r = wvT[:].rearrange("(dt p) h -> p dt h", p=P)

    with TileContext(nc) as tc:
        with (
            tc.tile_pool(name="persist", bufs=1) as persist,
            tc.tile_pool(name="consts", bufs=1) as consts,
        ):
            kt_sb = persist.tile([P, DT, KL], BF16, tag="kt")    # K^T[h%128, ht, k']
            v_sb = persist.tile([P, KT, D + 1], BF16, tag="v")   # V[k'%128, kt, h|1]
            qt_sb = persist.tile([P, DT, QL], BF16, tag="qt")    # Q^T[h%128, ht, q]

            bqp_sb = consts.tile([P, DT], F32, tag="bqp")
            bkp_sb = consts.tile([P, DT], F32, tag="bkp")
            bv_sb = consts.tile([P, D], F32, tag="bv")
            # biases on the ACT HWDGE queue; x chunks go on SP's -> they overlap
            nc.scalar.dma_start(out=bqp_sb[:], in_=bqp[:])
            nc.scalar.dma_start(out=bkp_sb[:], in_=bkp[:])
            # broadcast bv across all partitions (stride-0 partition AP -> SWDGE)
            bv_bcast = bass.AP(tensor=bv[:].tensor, offset=0, ap=[[0, P], [1, D]])
            nc.gpsimd.dma_start(out=bv_sb[:], in_=bv_bcast)

            # ---------------- phase 1: projections ----------------
            with (
                tc.tile_pool(name="wpool", bufs=3) as wpool,
                tc.tile_pool(name="xpool", bufs=3) as xpool,
                tc.tile_pool(name="projp", bufs=3, space="PSUM") as projp,
            ):
                # V first: its opening accumulation group only needs ONE
                # 512-col half of Wv plus a small first x chunk, so the PE
                # starts ~2x sooner after the DMA preamble than K would
                # (K's first group needs all of Wk).
                # V: out[s-tile, h-chunk] = sum_dt xvT[d,s-tile]^T @ WvT[d,h-chunk]
                # + bv (broadcast over rows), fused into the PSUM->SBUF move.
                w = wpool.tile([P, DT, D], BF16, tag="w")
                for hc in range(D // 512):
                    for dt in range(DT):
                        nc.scalar.dma_start(
                            out=w[:, dt, hc * 512:(hc + 1) * 512],
                            in_=wv_r[:, dt, hc * 512:(hc + 1) * 512],
                        )
                v_chunks = [(0, 128), (128, 384), (512, 512), (1024, 512), (1536, 512)]
                for c0, cw in v_chunks:
                    xc = xpool.tile([P, DT, XCH], BF16, tag="x")
                    nc.sync.dma_start(out=xc[:, :, :cw], in_=xv_r[:, :, c0:c0 + cw])
                    for st4 in range(cw // P):
                        st = c0 // P + st4
                        for hc in range(D // 512):
                            ps = projp.tile([P, 512], F32, tag="proj")
                            for dt in range(DT):
                                nc.tensor.matmul(
                                    ps[:],
                                    lhsT=xc[:, dt, st4 * P:(st4 + 1) * P],
                                    rhs=w[:, dt, hc * 512:(hc + 1) * 512],
                                    start=(dt == 0),
                                    stop=(dt == DT - 1),
                                )
                            nc.any.tensor_add(
                                out=v_sb[:, st, hc * 512:(hc + 1) * 512],
                                in0=ps[:],
                                in1=bv_sb[:, hc * 512:(hc + 1) * 512],
                            )
                nc.vector.memset(v_sb[:, :, D], 1.0)  # ones column -> row sums

                # K^T: out[h-tile, k'-chunk] = sum_dt WkT[d,h-tile]^T @ xkT[d,k'-chunk]
                w = wpool.tile([P, DT, D], BF16, tag="w")
                nc.scalar.dma_start(out=w[:], in_=wk_r)
                for cc in range(KL // XCH):
                    xc = xpool.tile([P, DT, XCH], BF16, tag="x")
                    nc.sync.dma_start(out=xc[:], in_=xk_r[:, :, cc * XCH:(cc + 1) * XCH])
                    for ht in range(DT):
                        ps = projp.tile([P, XCH], F32, tag="proj")
                        for dt in range(DT):
                            nc.tensor.matmul(
                                ps[:],
                                lhsT=w[:, dt, ht * P:(ht + 1) * P],
                                rhs=xc[:, dt, :],
                                start=(dt == 0),
                                stop=(dt == DT - 1),
                            )
                        nc.any.tensor_scalar_add(
                            out=kt_sb[:, ht, cc * XCH:(cc + 1) * XCH],
                            in0=ps[:],
                            scalar1=bkp_sb[:, ht:ht + 1],
                        )

                # Q^T: like K^T
                w = wpool.tile([P, DT, D], BF16, tag="w")
                nc.scalar.dma_start(out=w[:], in_=wq_r)
                for cc in range(QL // XCH):
                    xc = xpool.tile([P, DT, XCH], BF16, tag="x")
                    nc.sync.dma_start(out=xc[:], in_=xq_r[:, :, cc * XCH:(cc + 1) * XCH])
                    for ht in range(DT):
                        ps = projp.tile([P, XCH], F32, tag="proj")
                        for dt in range(DT):
                            nc.tensor.matmul(
                                ps[:],
                                lhsT=w[:, dt, ht * P:(ht + 1) * P],
                                rhs=xc[:, dt, :],
                                start=(dt == 0),
                                stop=(dt == DT - 1),
                            )
                        nc.any.tensor_scalar_add(
                            out=qt_sb[:, ht, cc * XCH:(cc + 1) * XCH],
                            in0=ps[:],
                            scalar1=bqp_sb[:, ht:ht + 1],
                        )

            # ---------------- phase 2: attention ----------------
            with (
                tc.tile_pool(name="ptpool", bufs=2) as ptpool,
                tc.tile_pool(name="opool", bufs=3) as opool,
                tc.tile_pool(name="small", bufs=4) as small,
                tc.tile_pool(name="scorep", bufs=2, space="PSUM") as scorep,
                tc.tile_pool(name="avp", bufs=4, space="PSUM") as avp,
            ):
                for qb in range(QL // QB):
                    q0 = qb * QB
                    ptb = ptpool.tile([P, KT, QB], BF16, tag="pt")
                    # scores S^T[k', q] for two k'-tiles at a time
                    for kp in range(KT // 2):
                        sp = scorep.tile([P, 2 * QB], F32, tag="score")
                        for half in range(2):
                            kt = kp * 2 + half
                            for ht in range(DT):
                                nc.tensor.matmul(
                                    sp[:, half * QB:(half + 1) * QB],
                                    lhsT=kt_sb[:, ht, kt * P:(kt + 1) * P],
                                    rhs=qt_sb[:, ht, q0:q0 + QB],
                                    start=(ht == 0),
                                    stop=(ht == DT - 1),
                                )
                        nc.scalar.activation(
                            out=ptb[:, kp * 2:(kp + 1) * 2, :].rearrange("p a b -> p (a b)"),
                            in_=sp[:],
                            func=mybir.ActivationFunctionType.Exp,
                            scale=SCALE,
                        )
                    # AV + row sums + normalize, one q-tile (128 rows) at a time.
                    # kt outer / chunk inner: the stationary (P^T tile) is
                    # reused across the 3 V chunks -> 1/3 the LDWEIGHTS.
                    for qt4 in range(QB // P):
                        qrow = q0 + qt4 * P
                        rl = small.tile([P, 1], F32, tag="rl")
                        ob = opool.tile([P, D], F32, tag="o")
                        for ci, (h0, h1) in enumerate(AV_CHUNKS):
                            av = avp.tile([P, AV_MAXW], F32, tag="av")
                            for kt in range(KT):
                                nc.tensor.matmul(
                                    av[:, :h1 - h0],
                                    lhsT=ptb[:, kt, qt4 * P:(qt4 + 1) * P],
                                    rhs=v_sb[:, kt, h0:h1],
                                    start=(kt == 0),
                                    stop=(kt == KT - 1),
                                )
                            if ci == 0:
                                # l (row sums) is the last column (global idx D)
                                nc.vector.reciprocal(rl[:], av[:, D - h0:D - h0 + 1])
                            w_ = min(h1, D) - h0
                            nc.any.tensor_scalar_mul(
                                out=ob[:, h0:h0 + w_],
                                in0=av[:, :w_],
                                scalar1=rl[:],
                            )
                            if qb == QL // QB - 1 and qt4 == QB // P - 1:
                                # very last q-tile: stream the output per chunk
                                # so the final DMA isn't serialized behind all
                                # three normalizes (shaves the tail barrier)
                                nc.sync.dma_start(
                                    out=out[qrow:qrow + P, h0:h0 + w_],
                                    in_=ob[:, h0:h0 + w_],
                                )
                        if not (qb == QL // QB - 1 and qt4 == QB // P - 1):
                            nc.sync.dma_start(out=out[qrow:qrow + P, :], in_=ob[:])

    nc.finalize()
    return nc


def prepare_in_maps(q_embd, k_embd, v_embd, Wq, bq, Wk, bk, Wv, bv):
    bf16 = ml_dtypes.bfloat16
    f32 = np.float32

    def t_cast(x):  # [B, L, D] -> [B, D, L] bf16
        return np.ascontiguousarray(np.swapaxes(np.asarray(x, f32), 1, 2)).astype(bf16)

    xqT = t_cast(q_embd)
    xkT = t_cast(k_embd)
    xvT = t_cast(v_embd)
    wqT = np.ascontiguousarray(np.asarray(Wq, f32).T).astype(bf16)
    wkT = np.ascontiguousarray(np.asarray(Wk, f32).T).astype(bf16)
    wvT = np.ascontiguousarray(np.asarray(Wv, f32).T).astype(bf16)
    bqp = np.ascontiguousarray(np.asarray(bq, f32).reshape(DT, P).T)
    bkp = np.ascontiguousarray(np.asarray(bk, f32).reshape(DT, P).T)
    bv_ = np.ascontiguousarray(np.asarray(bv, f32))

    return [
        {
            "xqT": xqT[i], "xkT": xkT[i], "xvT": xvT[i],
            "wqT": wqT, "wkT": wkT, "wvT": wvT,
            "bqp": bqp, "bkp": bkp, "bv": bv_,
        }
        for i in range(NCORES)
    ]


_NC_CACHE = None


def get_nc() -> bass.Bass:
    global _NC_CACHE
    if _NC_CACHE is None:
        _NC_CACHE = build_bass()
    return _NC_CACHE


def run_on_device(in_maps, trace=False, **kwargs):
    return run_bass_kernel_spmd(get_nc(), in_maps, list(range(NCORES)), trace=trace, **kwargs)


def kernel(q_embd, k_embd, v_embd, Wq, bq, Wk, bk, Wv, bv):
    in_maps = prepare_in_maps(q_embd, k_embd, v_embd, Wq, bq, Wk, bk, Wv, bv)
    res = run_on_device(in_maps)
    return np.stack([r["out"] for r in res.results], axis=0)



# revision 5
# speedup vs baseline: 1.1793x; 1.1793x over previous
"""Fused multi-head-size-1 attention kernel for Trainium2 (Bass/Tile).

Problem: out = softmax((x_q Wq^T + bq)(x_k Wk^T + bk)^T / sqrt(D)) (x_v Wv^T + bv)
Shapes: B=8, QL=KL=2048, D=1024, fp32 in/out.

Sharding: data-parallel over batch. Core i processes batch i end-to-end;
no collectives. Host pre-transposes x/W to contraction-major layout and
casts matmul operands to bf16 (PE runs bf16 at 1 cycle/row vs 4 for fp32;
all accumulation stays fp32 in PSUM).

Per-core dataflow (everything resident in SBUF in bf16):
  phase 1: K^T[h,k'] = Wk @ xk^T (+bk), V[k',h] = xv @ Wv^T (ones col
           appended for the softmax denominator), Q^T[h,q] = Wq @ xq^T (+bq)
  phase 2: per q-block: S^T[k',q] = K Q^T (PSUM, fp32), P^T = exp(S^T/32)
           (ScalarE, bf16 out), O[q,h] (+l) = P V_aug (PSUM, fp32),
           O = O * (1/l) + bv, DMA out.
"""

import numpy as np
import ml_dtypes

import concourse.bass as bass
import concourse.mybir as mybir
from concourse.bacc import Bacc
from concourse.tile import TileContext
from concourse.bass_utils import run_bass_kernel_spmd

B, QL, KL, D = 8, 2048, 2048, 1024
P = 128
NCORES = 8
DT = D // P          # 8 tiles along d/h
KT = KL // P         # 16 tiles along k'
XCH = 512            # x streaming chunk along s
QB = 512             # q block for the attention stage
F32 = mybir.dt.float32
BF16 = mybir.dt.bfloat16
SCALE = 1.0 / 32.0   # 1/sqrt(D)

# AV free-dim chunking over V's 1025 columns (1024 h + ones column for l).
# The l-carrying chunk goes first so the reciprocal overlaps the other
# chunks' matmuls.
AV_CHUNKS = [(684, 1025), (0, 342), (342, 684)]
AV_MAXW = 342


def build_bass() -> bass.Bass:
    # Bacc (not bare Bass): its finalize() runs the pass pipeline that splits
    # multi-semaphore waits into event semaphores (TRN2 allows 1 wait/inst).
    nc = Bacc()

    xqT = nc.declare_dram_parameter("xqT", [D, QL], BF16, isOutput=False)
    xkT = nc.declare_dram_parameter("xkT", [D, KL], BF16, isOutput=False)
    xvT = nc.declare_dram_parameter("xvT", [D, KL], BF16, isOutput=False)
    wqT = nc.declare_dram_parameter("wqT", [D, D], BF16, isOutput=False)
    wkT = nc.declare_dram_parameter("wkT", [D, D], BF16, isOutput=False)
    wvT = nc.declare_dram_parameter("wvT", [D, D], BF16, isOutput=False)
    bqp = nc.declare_dram_parameter("bqp", [P, DT], F32, isOutput=False)
    bkp = nc.declare_dram_parameter("bkp", [P, DT], F32, isOutput=False)
    bv = nc.declare_dram_parameter("bv", [D], F32, isOutput=False)
    out = nc.declare_dram_parameter("out", [QL, D], F32, isOutput=True)

    # contraction-major views: d = dt*128 + p
    xq_r = xqT[:].rearrange("(dt p) s -> p dt s", p=P)
    xk_r = xkT[:].rearrange("(dt p) s -> p dt s", p=P)
    xv_r = xvT[:].rearrange("(dt p) s -> p dt s", p=P)
    wq_r = wqT[:].rearrange("(dt p) h -> p dt h", p=P)
    wk_r = wkT[:].rearrange("(dt p) h -> p dt h", p=P)
    wv_r = wvT[:].rearrange("(dt p) h -> p dt h", p=P)

    with TileContext(nc) as tc:
        with (
            tc.tile_pool(name="persist", bufs=1) as persist,
            tc.tile_pool(name="consts", bufs=1) as consts,
        ):
            kt_sb = persist.tile([P, DT, KL], BF16, tag="kt")    # K^T[h%128, ht, k']
            v_sb = persist.tile([P, KT, D + 1], BF16, tag="v")   # V[k'%128, kt, h|1]
            qt_sb = persist.tile([P, DT, QL], BF16, tag="qt")    # Q^T[h%128, ht, q]

            bqp_sb = consts.tile([P, DT], F32, tag="bqp")
            bkp_sb = consts.tile([P, DT], F32, tag="bkp")
            bv_sb = consts.tile([P, D], F32, tag="bv")
            # broadcast bv across all partitions (stride-0 partition AP -> SWDGE)
            bv_bcast = bass.AP(tensor=bv[:].tensor, offset=0, ap=[[0, P], [1, D]])
            nc.gpsimd.dma_start(out=bv_sb[:], in_=bv_bcast)

            # ---------------- phase 1: projections ----------------
            with (
                tc.tile_pool(name="wpool", bufs=3) as wpool,
                tc.tile_pool(name="xpool", bufs=3) as xpool,
                tc.tile_pool(name="projp", bufs=6, space="PSUM") as projp,
            ):
                # V first: its opening accumulation group only needs ONE
                # 512-col half of Wv plus a small first x chunk, so the PE
                # starts ~2x sooner after the DMA preamble than K would
                # (K's first group needs all of Wk).
                # Each dma_start costs ~0.8-1us of descriptor-gen (DIRECT2D)
                # on the issuing sequencer, so the two Wv halves go as ONE
                # call each, split across the two HWDGE rings (ACT + SP) so
                # they transfer in parallel with the first x chunk.
                # V: out[s-tile, h-chunk] = sum_dt xvT[d,s-tile]^T @ WvT[d,h-chunk]
                # + bv (broadcast over rows), fused into the PSUM->SBUF move.
                w = wpool.tile([P, DT, D], BF16, tag="w")
                nc.scalar.dma_start(out=w[:, :, 0:512], in_=wv_r[:, :, 0:512])
                v_chunks = [(0, 128), (128, 384), (512, 512), (1024, 512), (1536, 512)]
                first_v = True
                for c0, cw in v_chunks:
                    xc = xpool.tile([P, DT, XCH], BF16, tag="x")
                    nc.sync.dma_start(out=xc[:, :, :cw], in_=xv_r[:, :, c0:c0 + cw])
                    if first_v:
                        # second Wv half rides the SP ring right behind the
                        # small first x chunk
                        nc.sync.dma_start(out=w[:, :, 512:1024], in_=wv_r[:, :, 512:1024])
                        first_v = False
                    for st4 in range(cw // P):
                        st = c0 // P + st4
                        for hc in range(D // 512):
                            ps = projp.tile([P, 512], F32, tag="proj")
                            for dt in range(DT):
                                nc.tensor.matmul(
                                    ps[:],
                                    lhsT=xc[:, dt, st4 * P:(st4 + 1) * P],
                                    rhs=w[:, dt, hc * 512:(hc + 1) * 512],
                                    start=(dt == 0),
                                    stop=(dt == DT - 1),
                                )
                            nc.any.tensor_add(
                                out=v_sb[:, st, hc * 512:(hc + 1) * 512],
                                in0=ps[:],
                                in1=bv_sb[:, hc * 512:(hc + 1) * 512],
                            )
                nc.vector.memset(v_sb[:, :, D], 1.0)  # ones column -> row sums

                # K^T: out[h-tile, k'-chunk] = sum_dt WkT[d,h-tile]^T @ xkT[d,k'-chunk]
                w = wpool.tile([P, DT, D], BF16, tag="w")
                nc.scalar.dma_start(out=w[:], in_=wk_r)
                # biases issue behind the weights on the ACT ring: ~1us of
                # descriptor-gen each, needed only at bias-add time
                nc.scalar.dma_start(out=bkp_sb[:], in_=bkp[:])
                nc.scalar.dma_start(out=bqp_sb[:], in_=bqp[:])
                for cc in range(KL // XCH):
                    xc = xpool.tile([P, DT, XCH], BF16, tag="x")
                    nc.sync.dma_start(out=xc[:], in_=xk_r[:, :, cc * XCH:(cc + 1) * XCH])
                    for ht in range(DT):
                        ps = projp.tile([P, XCH], F32, tag="proj")
                        for dt in range(DT):
                            nc.tensor.matmul(
                                ps[:],
                                lhsT=w[:, dt, ht * P:(ht + 1) * P],
                                rhs=xc[:, dt, :],
                                start=(dt == 0),
                                stop=(dt == DT - 1),
                            )
                        nc.any.tensor_scalar_add(
                            out=kt_sb[:, ht, cc * XCH:(cc + 1) * XCH],
                            in0=ps[:],
                            scalar1=bkp_sb[:, ht:ht + 1],
                        )

                # Q^T: like K^T
                w = wpool.tile([P, DT, D], BF16, tag="w")
                nc.scalar.dma_start(out=w[:], in_=wq_r)
                for cc in range(QL // XCH):
                    xc = xpool.tile([P, DT, XCH], BF16, tag="x")
                    nc.sync.dma_start(out=xc[:], in_=xq_r[:, :, cc * XCH:(cc + 1) * XCH])
                    for ht in range(DT):
                        ps = projp.tile([P, XCH], F32, tag="proj")
                        for dt in range(DT):
                            nc.tensor.matmul(
                                ps[:],
                                lhsT=w[:, dt, ht * P:(ht + 1) * P],
                                rhs=xc[:, dt, :],
                                start=(dt == 0),
                                stop=(dt == DT - 1),
                            )
                        nc.any.tensor_scalar_add(
                            out=qt_sb[:, ht, cc * XCH:(cc + 1) * XCH],
                            in0=ps[:],
                            scalar1=bqp_sb[:, ht:ht + 1],
                        )

            # ---------------- phase 2: attention ----------------
            with (
                tc.tile_pool(name="ptpool", bufs=2) as ptpool,
                tc.tile_pool(name="opool", bufs=3) as opool,
                tc.tile_pool(name="small", bufs=4) as small,
                tc.tile_pool(name="scorep", bufs=2, space="PSUM") as scorep,
                tc.tile_pool(name="avp", bufs=4, space="PSUM") as avp,
            ):
                for qb in range(QL // QB):
                    q0 = qb * QB
                    ptb = ptpool.tile([P, KT, QB], BF16, tag="pt")
                    # scores S^T[k', q] for two k'-tiles at a time
                    for kp in range(KT // 2):
                        sp = scorep.tile([P, 2 * QB], F32, tag="score")
                        for half in range(2):
                            kt = kp * 2 + half
                            for ht in range(DT):
                                nc.tensor.matmul(
                                    sp[:, half * QB:(half + 1) * QB],
                                    lhsT=kt_sb[:, ht, kt * P:(kt + 1) * P],
                                    rhs=qt_sb[:, ht, q0:q0 + QB],
                                    start=(ht == 0),
                                    stop=(ht == DT - 1),
                                )
                        nc.scalar.activation(
                            out=ptb[:, kp * 2:(kp + 1) * 2, :].rearrange("p a b -> p (a b)"),
                            in_=sp[:],
                            func=mybir.ActivationFunctionType.Exp,
                            scale=SCALE,
                        )
                    # AV + row sums + normalize, one q-tile (128 rows) at a time.
                    # kt outer / chunk inner: the stationary (P^T tile) is
                    # reused across the 3 V chunks -> 1/3 the LDWEIGHTS.
                    for qt4 in range(QB // P):
                        qrow = q0 + qt4 * P
                        rl = small.tile([P, 1], F32, tag="rl")
                        ob = opool.tile([P, D], F32, tag="o")
                        for ci, (h0, h1) in enumerate(AV_CHUNKS):
                            av = avp.tile([P, AV_MAXW], F32, tag="av")
                            for kt in range(KT):
                                nc.tensor.matmul(
                                    av[:, :h1 - h0],
                                    lhsT=ptb[:, kt, qt4 * P:(qt4 + 1) * P],
                                    rhs=v_sb[:, kt, h0:h1],
                                    start=(kt == 0),
                                    stop=(kt == KT - 1),
                                )
                            if ci == 0:
                                # l (row sums) is the last column (global idx D)
                                nc.vector.reciprocal(rl[:], av[:, D - h0:D - h0 + 1])
                            w_ = min(h1, D) - h0
                            nc.any.tensor_scalar_mul(
                                out=ob[:, h0:h0 + w_],
                                in0=av[:, :w_],
                                scalar1=rl[:],
                            )
                            if qb == QL // QB - 1 and qt4 == QB // P - 1:
                                # very last q-tile: stream the output per chunk
                                # so the final DMA isn't serialized behind all
                                # three normalizes (shaves the tail barrier)
                                nc.sync.dma_start(
                                    out=out[qrow:qrow + P, h0:h0 + w_],
                                    in_=ob[:, h0:h0 + w_],
                                )
                        if not (qb == QL // QB - 1 and qt4 == QB // P - 1):
                            nc.sync.dma_start(out=out[qrow:qrow + P, :], in_=ob[:])

    nc.finalize()
    return nc


def prepare_in_maps(q_embd, k_embd, v_embd, Wq, bq, Wk, bk, Wv, bv):
    bf16 = ml_dtypes.bfloat16
    f32 = np.float32

    def t_cast(x):  # [B, L, D] -> [B, D, L] bf16
        return np.ascontiguousarray(np.swapaxes(np.asarray(x, f32), 1, 2)).astype(bf16)

    xqT = t_cast(q_embd)
    xkT = t_cast(k_embd)
    xvT = t_cast(v_embd)
    wqT = np.ascontiguousarray(np.asarray(Wq, f32).T).astype(bf16)
    wkT = np.ascontiguousarray(np.asarray(Wk, f32).T).astype(bf16)
    wvT = np.ascontiguousarray(np.asarray(Wv, f32).T).astype(bf16)
    bqp = np.ascontiguousarray(np.asarray(bq, f32).reshape(DT, P).T)
    bkp = np.ascontiguousarray(np.asarray(bk, f32).reshape(DT, P).T)
    bv_ = np.ascontiguousarray(np.asarray(bv, f32))

    return [
        {
            "xqT": xqT[i], "xkT": xkT[i], "xvT": xvT[i],
            "wqT": wqT, "wkT": wkT, "wvT": wvT,
            "bqp": bqp, "bkp": bkp, "bv": bv_,
        }
        for i in range(NCORES)
    ]


_NC_CACHE = None


def get_nc() -> bass.Bass:
    global _NC_CACHE
    if _NC_CACHE is None:
        _NC_CACHE = build_bass()
    return _NC_CACHE


def run_on_device(in_maps, trace=False, **kwargs):
    return run_bass_kernel_spmd(get_nc(), in_maps, list(range(NCORES)), trace=trace, **kwargs)


def kernel(q_embd, k_embd, v_embd, Wq, bq, Wk, bk, Wv, bv):
    in_maps = prepare_in_maps(q_embd, k_embd, v_embd, Wq, bq, Wk, bk, Wv, bv)
    res = run_on_device(in_maps)
    return np.stack([r["out"] for r in res.results], axis=0)



# revision 12
# speedup vs baseline: 1.2047x; 1.0216x over previous
"""Fused multi-head-size-1 attention kernel for Trainium2 (Bass/Tile).

Problem: out = softmax((x_q Wq^T + bq)(x_k Wk^T + bk)^T / sqrt(D)) (x_v Wv^T + bv)
Shapes: B=8, QL=KL=2048, D=1024, fp32 in/out.

Sharding: data-parallel over batch. Core i processes batch i end-to-end;
no collectives. Host pre-transposes x/W to contraction-major layout and
casts matmul operands to bf16 (PE runs bf16 at 1 cycle/row vs 4 for fp32;
all accumulation stays fp32 in PSUM).

Per-core dataflow (everything resident in SBUF in bf16):
  phase 1: K^T[h,k'] = Wk @ xk^T (+bk), V[k',h] = xv @ Wv^T (ones col
           appended for the softmax denominator), Q^T[h,q] = Wq @ xq^T (+bq)
  phase 2: per q-block: S^T[k',q] = K Q^T (PSUM, fp32), P^T = exp(S^T/32)
           (ScalarE, bf16 out), O[q,h] (+l) = P V_aug (PSUM, fp32),
           O = O * (1/l) + bv, DMA out.
"""

import numpy as np
import ml_dtypes

import concourse.bass as bass
import concourse.mybir as mybir
from concourse.bacc import Bacc
from concourse.tile import TileContext
from concourse.bass_utils import run_bass_kernel_spmd

B, QL, KL, D = 8, 2048, 2048, 1024
P = 128
NCORES = 8
DT = D // P          # 8 tiles along d/h
KT = KL // P         # 16 tiles along k'
XCH = 512            # x streaming chunk along s
QB = 512             # q block for the attention stage
F32 = mybir.dt.float32
BF16 = mybir.dt.bfloat16
SCALE = 1.0 / 32.0   # 1/sqrt(D)

# AV free-dim chunking over V's 1025 columns (1024 h + ones column for l).
# The l-carrying chunk goes first so the reciprocal overlaps the other
# chunks' matmuls.
AV_CHUNKS = [(684, 1025), (0, 342), (342, 684)]
AV_MAXW = 342


def build_bass() -> bass.Bass:
    # Bacc (not bare Bass): its finalize() runs the pass pipeline that splits
    # multi-semaphore waits into event semaphores (TRN2 allows 1 wait/inst).
    nc = Bacc()

    xqT = nc.declare_dram_parameter("xqT", [D, QL], BF16, isOutput=False)
    xkT = nc.declare_dram_parameter("xkT", [D, KL], BF16, isOutput=False)
    xvT = nc.declare_dram_parameter("xvT", [D, KL], BF16, isOutput=False)
    wqT = nc.declare_dram_parameter("wqT", [D, D], BF16, isOutput=False)
    wkT = nc.declare_dram_parameter("wkT", [D, D], BF16, isOutput=False)
    wvT = nc.declare_dram_parameter("wvT", [D, D], BF16, isOutput=False)
    bqp = nc.declare_dram_parameter("bqp", [P, DT], F32, isOutput=False)
    bkp = nc.declare_dram_parameter("bkp", [P, DT], F32, isOutput=False)
    bvt = nc.declare_dram_parameter("bvt", [P, D], F32, isOutput=False)
    out = nc.declare_dram_parameter("out", [QL, D], F32, isOutput=True)

    # contraction-major views: d = dt*128 + p
    xq_r = xqT[:].rearrange("(dt p) s -> p dt s", p=P)
    xk_r = xkT[:].rearrange("(dt p) s -> p dt s", p=P)
    xv_r = xvT[:].rearrange("(dt p) s -> p dt s", p=P)
    wq_r = wqT[:].rearrange("(dt p) h -> p dt h", p=P)
    wk_r = wkT[:].rearrange("(dt p) h -> p dt h", p=P)
    wv_r = wvT[:].rearrange("(dt p) h -> p dt h", p=P)

    with TileContext(nc) as tc:
        with (
            tc.tile_pool(name="persist", bufs=1) as persist,
            tc.tile_pool(name="consts", bufs=1) as consts,
        ):
            kt_sb = persist.tile([P, DT, KL], BF16, tag="kt")    # K^T[h%128, ht, k']
            v_sb = persist.tile([P, KT, D + 1], BF16, tag="v")   # V[k'%128, kt, h|1]
            qt_sb = persist.tile([P, DT, QL], BF16, tag="qt")    # Q^T[h%128, ht, q]

            bqp_sb = consts.tile([P, DT], F32, tag="bqp")
            bkp_sb = consts.tile([P, DT], F32, tag="bkp")
            bv_sb = consts.tile([P, D], F32, tag="bv")

            # ---------------- phase 1: projections ----------------
            with (
                tc.tile_pool(name="wpool", bufs=3) as wpool,
                tc.tile_pool(name="xpool", bufs=3) as xpool,
                tc.tile_pool(name="projp", bufs=6, space="PSUM") as projp,
            ):
                # V first: its opening accumulation group only needs ONE
                # 512-col half of Wv plus a small first x chunk, so the PE
                # starts ~2x sooner after the DMA preamble than K would
                # (K's first group needs all of Wk).
                # DMA ring schedule: the two HWDGE rings are FIFO per ring and
                # each dma_start costs ~0.8us of descriptor-gen on its
                # sequencer, so the critical-path transfers must lead their
                # ring and nothing else may compete in the first ~20us:
                #   ACT ring (scalar): Wv-hc0 | xv1..4 | bkp bqp | xk* | xq*
                #   SP  ring (sync):   xv0 | Wv-hc1 | bv | Wk | Wq | out stores
                # V: out[s-tile, h-chunk] = sum_dt xvT[d,s-tile]^T @ WvT[d,h-chunk]
                # + bv (broadcast over rows), fused into the PSUM->SBUF move.
                w = wpool.tile([P, DT, D], BF16, tag="w")
                nc.scalar.dma_start(out=w[:, :, 0:512], in_=wv_r[:, :, 0:512])
                v_chunks = [(0, 128), (128, 384), (512, 512), (1024, 512), (1536, 512)]
                first_v = True
                for c0, cw in v_chunks:
                    xc = xpool.tile([P, DT, XCH], BF16, tag="x")
                    if first_v:
                        nc.sync.dma_start(out=xc[:, :, :cw], in_=xv_r[:, :, c0:c0 + cw])
                        # second Wv half + bv ride the SP ring right behind
                        # the small first x chunk
                        nc.sync.dma_start(out=w[:, :, 512:1024], in_=wv_r[:, :, 512:1024])
                        nc.sync.dma_start(out=bv_sb[:], in_=bvt[:])
                        first_v = False
                    else:
                        nc.scalar.dma_start(out=xc[:, :, :cw], in_=xv_r[:, :, c0:c0 + cw])
                    for st4 in range(cw // P):
                        st = c0 // P + st4
                        for hc in range(D // 512):
                            ps = projp.tile([P, 512], F32, tag="proj")
                            for dt in range(DT):
                                nc.tensor.matmul(
                                    ps[:],
                                    lhsT=xc[:, dt, st4 * P:(st4 + 1) * P],
                                    rhs=w[:, dt, hc * 512:(hc + 1) * 512],
                                    start=(dt == 0),
                                    stop=(dt == DT - 1),
                                )
                            nc.vector.tensor_add(
                                out=v_sb[:, st, hc * 512:(hc + 1) * 512],
                                in0=ps[:],
                                in1=bv_sb[:, hc * 512:(hc + 1) * 512],
                            )
                nc.vector.memset(v_sb[:, :, D], 1.0)  # ones column -> row sums

                # K^T: out[h-tile, k'-chunk] = sum_dt WkT[d,h-tile]^T @ xkT[d,k'-chunk]
                w = wpool.tile([P, DT, D], BF16, tag="w")
                nc.sync.dma_start(out=w[:], in_=wk_r)
                # biases ride the ACT ring behind the V x chunks; needed only
                # at bias-add time (~90us+)
                nc.scalar.dma_start(out=bkp_sb[:], in_=bkp[:])
                nc.scalar.dma_start(out=bqp_sb[:], in_=bqp[:])
                for cc in range(KL // XCH):
                    xc = xpool.tile([P, DT, XCH], BF16, tag="x")
                    nc.scalar.dma_start(out=xc[:], in_=xk_r[:, :, cc * XCH:(cc + 1) * XCH])
                    for ht in range(DT):
                        ps = projp.tile([P, XCH], F32, tag="proj")
                        for dt in range(DT):
                            nc.tensor.matmul(
                                ps[:],
                                lhsT=w[:, dt, ht * P:(ht + 1) * P],
                                rhs=xc[:, dt, :],
                                start=(dt == 0),
                                stop=(dt == DT - 1),
                            )
                        nc.vector.tensor_scalar_add(
                            out=kt_sb[:, ht, cc * XCH:(cc + 1) * XCH],
                            in0=ps[:],
                            scalar1=bkp_sb[:, ht:ht + 1],
                        )

                # Q^T: like K^T
                w = wpool.tile([P, DT, D], BF16, tag="w")
                nc.sync.dma_start(out=w[:], in_=wq_r)
                for cc in range(QL // XCH):
                    xc = xpool.tile([P, DT, XCH], BF16, tag="x")
                    nc.scalar.dma_start(out=xc[:], in_=xq_r[:, :, cc * XCH:(cc + 1) * XCH])
                    for ht in range(DT):
                        ps = projp.tile([P, XCH], F32, tag="proj")
                        for dt in range(DT):
                            nc.tensor.matmul(
                                ps[:],
                                lhsT=w[:, dt, ht * P:(ht + 1) * P],
                                rhs=xc[:, dt, :],
                                start=(dt == 0),
                                stop=(dt == DT - 1),
                            )
                        nc.vector.tensor_scalar_add(
                            out=qt_sb[:, ht, cc * XCH:(cc + 1) * XCH],
                            in0=ps[:],
                            scalar1=bqp_sb[:, ht:ht + 1],
                        )

            # ---------------- phase 2: attention ----------------
            with (
                tc.tile_pool(name="ptpool", bufs=2) as ptpool,
                tc.tile_pool(name="opool", bufs=3) as opool,
                tc.tile_pool(name="small", bufs=4) as small,
                tc.tile_pool(name="scorep", bufs=2, space="PSUM") as scorep,
                tc.tile_pool(name="avp", bufs=4, space="PSUM") as avp,
            ):
                for qb in range(QL // QB):
                    q0 = qb * QB
                    ptb = ptpool.tile([P, KT, QB], BF16, tag="pt")
                    # scores S^T[k', q] for two k'-tiles at a time
                    for kp in range(KT // 2):
                        sp = scorep.tile([P, 2 * QB], F32, tag="score")
                        for half in range(2):
                            kt = kp * 2 + half
                            for ht in range(DT):
                                nc.tensor.matmul(
                                    sp[:, half * QB:(half + 1) * QB],
                                    lhsT=kt_sb[:, ht, kt * P:(kt + 1) * P],
                                    rhs=qt_sb[:, ht, q0:q0 + QB],
                                    start=(ht == 0),
                                    stop=(ht == DT - 1),
                                )
                        nc.scalar.activation(
                            out=ptb[:, kp * 2:(kp + 1) * 2, :].rearrange("p a b -> p (a b)"),
                            in_=sp[:],
                            func=mybir.ActivationFunctionType.Exp,
                            scale=SCALE,
                        )
                    # AV + row sums + normalize, one q-tile (128 rows) at a time.
                    # kt outer / chunk inner: the stationary (P^T tile) is
                    # reused across the 3 V chunks -> 1/3 the LDWEIGHTS.
                    for qt4 in range(QB // P):
                        qrow = q0 + qt4 * P
                        rl = small.tile([P, 1], F32, tag="rl")
                        ob = opool.tile([P, D], F32, tag="o")
                        for ci, (h0, h1) in enumerate(AV_CHUNKS):
                            av = avp.tile([P, AV_MAXW], F32, tag="av")
                            for kt in range(KT):
                                nc.tensor.matmul(
                                    av[:, :h1 - h0],
                                    lhsT=ptb[:, kt, qt4 * P:(qt4 + 1) * P],
                                    rhs=v_sb[:, kt, h0:h1],
                                    start=(kt == 0),
                                    stop=(kt == KT - 1),
                                )
                            if ci == 0:
                                # l (row sums) is the last column (global idx D)
                                nc.vector.reciprocal(rl[:], av[:, D - h0:D - h0 + 1])
                            w_ = min(h1, D) - h0
                            nc.vector.tensor_scalar_mul(
                                out=ob[:, h0:h0 + w_],
                                in0=av[:, :w_],
                                scalar1=rl[:],
                            )
                            if qb == QL // QB - 1 and qt4 == QB // P - 1:
                                # very last q-tile: stream the output per chunk
                                # so the final DMA isn't serialized behind all
                                # three normalizes (shaves the tail barrier)
                                nc.sync.dma_start(
                                    out=out[qrow:qrow + P, h0:h0 + w_],
                                    in_=ob[:, h0:h0 + w_],
                                )
                        if not (qb == QL // QB - 1 and qt4 == QB // P - 1):
                            nc.sync.dma_start(out=out[qrow:qrow + P, :], in_=ob[:])

    nc.finalize()
    return nc


def prepare_in_maps(q_embd, k_embd, v_embd, Wq, bq, Wk, bk, Wv, bv):
    bf16 = ml_dtypes.bfloat16
    f32 = np.float32

    def t_cast(x):  # [B, L, D] -> [B, D, L] bf16
        return np.ascontiguousarray(np.swapaxes(np.asarray(x, f32), 1, 2)).astype(bf16)

    xqT = t_cast(q_embd)
    xkT = t_cast(k_embd)
    xvT = t_cast(v_embd)
    wqT = np.ascontiguousarray(np.asarray(Wq, f32).T).astype(bf16)
    wkT = np.ascontiguousarray(np.asarray(Wk, f32).T).astype(bf16)
    wvT = np.ascontiguousarray(np.asarray(Wv, f32).T).astype(bf16)
    bqp = np.ascontiguousarray(np.asarray(bq, f32).reshape(DT, P).T)
    bkp = np.ascontiguousarray(np.asarray(bk, f32).reshape(DT, P).T)
    bvt = np.ascontiguousarray(np.tile(np.asarray(bv, f32)[None, :], (P, 1)))

    return [
        {
            "xqT": xqT[i], "xkT": xkT[i], "xvT": xvT[i],
            "wqT": wqT, "wkT": wkT, "wvT": wvT,
            "bqp": bqp, "bkp": bkp, "bvt": bvt,
        }
        for i in range(NCORES)
    ]


_NC_CACHE = None


def get_nc() -> bass.Bass:
    global _NC_CACHE
    if _NC_CACHE is None:
        _NC_CACHE = build_bass()
    return _NC_CACHE


def run_on_device(in_maps, trace=False, **kwargs):
    return run_bass_kernel_spmd(get_nc(), in_maps, list(range(NCORES)), trace=trace, **kwargs)


def kernel(q_embd, k_embd, v_embd, Wq, bq, Wk, bk, Wv, bv):
    in_maps = prepare_in_maps(q_embd, k_embd, v_embd, Wq, bq, Wk, bk, Wv, bv)
    res = run_on_device(in_maps)
    return np.stack([r["out"] for r in res.results], axis=0)

